# revision 37
# baseline (speedup 1.0000x reference)
"""TRN2 Bass kernel: fused LSTM cell (nn_CustomLSTMCell), 8-core tensor-parallel.

Strategy
--------
gates = x @ W_ih.T + b_ih + h_prev @ W_hh.T + b_hh  is computed as ONE GEMM
with contraction K = I + H = 4096 over xh = [x | h_prev] and W = [W_ih | W_hh].

The 4H gate dimension is tensor-parallel sharded across the 8 cores: core c
owns h-columns [c*256, (c+1)*256) of every gate (i, f, g, o).  Each core
computes gatesT [1024, 2048] = Wc @ xh.T with gate rows on partitions, so the
per-gate bias is a native per-partition scalar in scalar.activation, which
also applies sigmoid/tanh while evicting PSUM -> SBUF.  The LSTM cell update
(new_C = f*C + i*g, new_h = o*tanh(new_C)) runs on the vector engine, fully
overlapped with the tensor engine.  No collectives: output slices are
disjoint and gathered on the host.

Matmul operands are fp16 (4x PE rate vs fp32, accumulation fp32 in PSUM).

DMA plan (the early phase is HBM-bandwidth critical: the first two matmul
groups consume W (8MB) + xh gen0 (4MB) in ~55us = ~290GB/s of the 358GB/s
HBM budget):
 - All DRAM arrays are pre-swizzled on the host so every DMA is a large
   per-partition-contiguous block (0.25-1MB, 2-8KB/partition/line): W and xh
   stream as chunk-group slabs with a geometric first ramp so the PE can
   start ~3us after the engines leave the framework preamble.
 - C is fp16 (1MB instead of 2MB of competing traffic), one DMA.
 - All 6 outputs are written fp16 into one staging tile per (n, hb) group
   and stored with a single 0.75MB DMA on the Scalar HWDGE queue (never
   GpSimd: SWDGE store drains cost ~8us at kernel end).
 - Final group is gate-major (f,i,g then o): everything except the o-gate
   and new_h columns is stored while the o-gate matmuls still run, so only
   0.25MB of stores remain after the last matmul.
 - Batch-tile n=0 runs chunk-major across BOTH hb halves (8 PSUM banks), so
   the 8MB of W spreads over 55us (74GB/s per W half) instead of two 4MB
   bursts -- the ramp phase is HBM/SDMA-engine limited at ~105GB/s/queue.
Queues: Sync = W-h0 + bias + C/2, Scalar = W-h1 + C/2 + stores, GpSimd = xh.
"""

import numpy as np

B = 2048           # batch
I_DIM = 2048       # input features
H = 2048           # hidden
NCORES = 8
S = H // NCORES    # 256: per-core h-slice (per gate)
M_PER_CORE = 4 * S # 1024 gate rows per core
K = I_DIM + H      # 4096 fused contraction dim
P = 128
KC = K // P        # 32 contraction chunks
NT = B // 512      # 4 batch tiles of 512
HB = S // P        # 2 h-blocks of 128 per core

# Chunk-group slabs per stream, geometric ramp: slab granularity trades
# first-MM latency (small early slabs) against completion lumpiness (a chunk
# waits for its WHOLE slab plus ~1-2us of DMA-completion-receipt latency).
# The ~4 DMA-completion sem lanes per HWDGE queue only delay late DISPATCHES,
# which stay ahead of the FIFO data flow, so fine slabs are safe.
GROUPS = [
    (0, 1), (1, 2), (2, 3), (3, 4), (4, 6), (6, 8),
    (8, 11), (11, 14), (14, 18), (18, 23), (23, 28), (28, 32),
]
CHUNK_GRP = {}
for _gi, (_s, _e) in enumerate(GROUPS):
    for _k in range(_s, _e):
        CHUNK_GRP[_k] = (_gi, _k - _s)

NWARM = 12         # cold matmuls x ~427ns bridge the preamble->first-slab gap
                   # (>=8 warms HAM; 12 delays the first real MM to ~12us,
                   # when the early slabs have actually landed)

_F16 = np.float16

# staging column order within the 3072-wide per-group output slab
J_F, J_I, J_G, J_CN, J_O, J_HN = range(6)

_CACHE = {}


def _build_program():
    from contextlib import ExitStack

    import concourse.mybir as mybir
    import concourse.tile as tile
    from concourse import bacc

    f32 = mybir.dt.float32
    f16 = mybir.dt.float16
    AF = mybir.ActivationFunctionType

    nc = bacc.Bacc("TRN2", target_bir_lowering=False, debug=False)

    # DRAM layouts (host pre-swizzled, every DMA per-partition contiguous):
    # w_t: row (hb*128 + p), col (k*512 + m').  A chunk-group slab (hb, ks:ke)
    # is w_t[hb*128:(hb+1)*128, ks*512:ke*512] -- contiguous per partition.
    w_t = nc.dram_tensor("w_t", [2 * P, KC * 512], f16, kind="ExternalInput")
    xh_t = nc.dram_tensor("xh_t", [NT * P, KC * 512], f16, kind="ExternalInput")
    # xh_t: row (n*128 + p), col (k*512 + cc).
    bias_d = nc.dram_tensor("bias", [P, 8], f32, kind="ExternalInput")
    c_t = nc.dram_tensor("c_t", [P, 2 * B], f16, kind="ExternalInput")
    # c_t: [p, hb*2048 + cc]
    out6 = nc.dram_tensor("out6", [NT * 2 * P, 6 * 512], f16, kind="ExternalOutput")
    # out6: row ((n*2 + hb)*128 + p), col j*512 + cc

    w_ap = w_t.ap()
    xh_ap = xh_t.ap()
    bias_ap = bias_d.ap()
    c_ap = c_t.ap()
    out_ap = out6.ap()

    # rearrange DRAM views so the partition and chunk dims are explicit:
    w_r = w_ap.rearrange("(h p) (a m) -> p h a m", p=P, m=512)   # [128,2,KC,512]
    xh_r = xh_ap.rearrange("(n p) (a m) -> n p a m", p=P, m=512)  # [4,128,KC,512]

    ACT_FN = [AF.Sigmoid, AF.Sigmoid, AF.Tanh, AF.Sigmoid]  # i, f, g, o

    with tile.TileContext(nc) as tc, ExitStack() as ctx:
        w_pool = ctx.enter_context(tc.tile_pool(name="w", bufs=1))
        xh_pool = ctx.enter_context(tc.tile_pool(name="xh", bufs=2))
        c_pool = ctx.enter_context(tc.tile_pool(name="c", bufs=1))
        b_pool = ctx.enter_context(tc.tile_pool(name="b", bufs=1))
        psum_pool = ctx.enter_context(tc.tile_pool(name="ps", bufs=2, space="PSUM"))
        st_pool = ctx.enter_context(tc.tile_pool(name="st", bufs=2))
        sc_pool = ctx.enter_context(tc.tile_pool(name="sc", bufs=2))

        # ---- preamble DMAs -------------------------------------------------
        # Queue plan (each queue drains ~1/3 of the ~310GB/s early HBM budget
        # when all are busy, so every queue's deadline-critical load must stay
        # under ~105GB/s): Sync = W-h0 stream + bias + C-half; Scalar = W-h1
        # stream + C-half + stores; GpSimd = xh generations.
        bias_all = b_pool.tile([P, 8], f32)

        # Warm-up matmuls on dummy data bridge the gap from the framework
        # preamble (~6us) to the first data slab (~10us) and pre-warm HAM.
        dummy = b_pool.tile([P, 512], f16)
        nc.vector.memset(dummy[:], 0.0)
        warm_ps = psum_pool.tile([P, 512], f32, name="ps0")
        for i in range(NWARM):
            nc.tensor.matmul(
                warm_ps[:], dummy[:, 0:P], dummy[:],
                start=(i == 0), stop=(i == NWARM - 1),
            )

        def w_slab(gi):
            # one tile holds both hb halves of a chunk group; the two halves
            # stream on different queues (disjoint ranges of the tile).
            ks, ke = GROUPS[gi]
            t = w_pool.tile([P, HB, ke - ks, 512], f16, name=f"wg{gi}")
            nc.sync.dma_start(t[:, 0, :, :], w_r[:, 0, ks:ke, :])
            nc.scalar.dma_start(t[:, 1, :, :], w_r[:, 1, ks:ke, :])
            return t

        def xh_slab(n, gi, eng=None):
            ks, ke = GROUPS[gi]
            t = xh_pool.tile([P, ke - ks, 512], f16, name=f"xh{gi}")
            (eng or nc.gpsimd).dma_start(t[:], xh_r[n, :, ks:ke, :])
            return t

        NG = len(GROUPS)
        w_tiles = [None] * NG
        xh_tiles = {}
        xh_tiles[0] = [None] * NG
        for gi in range(NG):
            w_tiles[gi] = w_slab(gi)
            # the very first xh slab rides sync (HWDGE ~0.6us first-byte vs
            # SWDGE ~1us) so the first real matmul starts sooner
            xh_tiles[0][gi] = xh_slab(0, gi, eng=nc.sync if gi == 0 else None)
        # Measured early fair shares under 3-queue contention: ~105GB/s per
        # queue (HBM-stack bound, all 8 cores ramping).  Streams need 74GB/s
        # each (w-h0 / w-h1 / xh0); bias + C (needed only by the first
        # epilogue at ~64us) are split across the two HWDGE queues so no
        # queue exceeds ~87GB/s of deadline load.
        nc.sync.dma_start(bias_all[:], bias_ap[:, :])
        c_all = c_pool.tile([P, 2 * B], f16)
        nc.sync.dma_start(c_all[:, 0:B], c_ap[:, 0:B])
        nc.scalar.dma_start(c_all[:, B : 2 * B], c_ap[:, B : 2 * B])

        # The 16 SDMA engines saturate at ~300GB/s aggregate (4KB packets,
        # ~20GB/s each) split round-robin-fairly per queue, so generation
        # n+1 must stay OFF the gpsimd queue until generation n has landed:
        # a tiny copy from gen n's last slab INTO each new slab tile makes
        # the new DMA (WAW) wait for gen n's completion, whatever order the
        # scheduler picks.
        def gate_then_prefetch(n):
            src = xh_tiles[n][NG - 1]
            xh_tiles[n + 1] = []
            for gi in range(NG):
                ks, ke = GROUPS[gi]
                t = xh_pool.tile([P, ke - ks, 512], f16, name=f"xh{gi}")
                nc.gpsimd.tensor_copy(t[:, 0, 0:4], src[:, 0, 0:4])
                nc.gpsimd.dma_start(t[:], xh_r[n + 1, :, ks:ke, :])
                xh_tiles[n + 1].append(t)

        gate_then_prefetch(0)

        # ---- main loop -----------------------------------------------------
        def lhs(hb, k, g):
            gi, off = CHUNK_GRP[k]
            return w_tiles[gi][:, hb, off, g * P : (g + 1) * P]

        def rhs(n, k):
            gi, off = CHUNK_GRP[k]
            return xh_tiles[n][gi][:, off, :]

        def cell_update(st, hb, n, sl):
            fc = sc_pool.tile([P, 512], f16, name="fc")
            nc.vector.tensor_mul(
                fc[:], sl(J_F), c_all[:, hb * B + n * 512 : hb * B + (n + 1) * 512]
            )
            ig = sc_pool.tile([P, 512], f16, name="ig")
            nc.vector.tensor_mul(ig[:], sl(J_I), sl(J_G))
            nc.vector.tensor_add(sl(J_CN), ig[:], fc[:])

        # --- n = 0: chunk-major over BOTH hb halves (all 8 PSUM banks).
        # This halves the early W bandwidth demand (the whole 8MB spreads
        # over 55us instead of landing in two 4MB/27us bursts) -- the ramp
        # phase is HBM-limited and this is what keeps the PE fed.
        pss = [
            [psum_pool.tile([P, 512], f32, name=f"ps{g}") for g in range(4)]
            for _hb in range(HB)
        ]
        # hb1's chunk loop runs SKEW chunks behind hb0's: w-h1 streams on the
        # scalar queue, whose early throughput lags sync by ~2-3us, and PSUM
        # accumulation order is free.  Bonus: hb0's banks close ~2.6us before
        # the last matmul, so their evictions fully hide.
        SKEW = 3
        for s in range(KC + SKEW):
            if s < KC:
                for g in range(4):
                    nc.tensor.matmul(
                        pss[0][g][:], lhs(0, s, g), rhs(0, s),
                        start=(s == 0), stop=(s == KC - 1),
                    )
            if s >= SKEW:
                k = s - SKEW
                for g in range(4):
                    nc.tensor.matmul(
                        pss[1][g][:], lhs(1, k, g), rhs(0, k),
                        start=(k == 0), stop=(k == KC - 1),
                    )
        # epilogue: evict hb0's banks first (they close first and the next
        # group's matmuls reuse them first), then hb1, then the cell chains.
        sts = [st_pool.tile([P, 6 * 512], f16, name="st") for _hb in range(HB)]
        sls = [
            (lambda st: lambda j: st[:, j * 512 : (j + 1) * 512])(st) for st in sts
        ]
        for hb in range(HB):
            for g, j in ((0, J_I), (1, J_F), (2, J_G), (3, J_O)):
                nc.scalar.activation(
                    sls[hb](j), pss[hb][g][:], ACT_FN[g],
                    bias=bias_all[:, hb * 4 + g : hb * 4 + g + 1],
                )
        for hb in range(HB):
            cell_update(sts[hb], hb, 0, sls[hb])
        for hb in range(HB):
            th = sc_pool.tile([P, 512], f16, name="th")
            nc.scalar.activation(th[:], sls[hb](J_CN), AF.Tanh)
            nc.vector.tensor_mul(sls[hb](J_HN), sls[hb](J_O), th[:])
            nc.scalar.dma_start(out_ap[hb * P : (hb + 1) * P, :], sts[hb][:])

        # --- n = 1..3: pairwise (n, hb) groups, 4 banks each (W resident,
        # xh prefetched a generation ahead -- DMA is far ahead by now).
        for n in range(1, NT):
            if n + 1 < NT:
                gate_then_prefetch(n)

            for hb in range(HB):
                final = n == NT - 1 and hb == HB - 1
                ps = [
                    psum_pool.tile([P, 512], f32, name=f"ps{g}") for g in range(4)
                ]
                if final:
                    # gate-major (f,i,g,o): f/i/g PSUMs close early so the
                    # cell-state chain + bulk store overlap the o matmuls.
                    for g in (1, 0, 2, 3):
                        for k in range(KC):
                            nc.tensor.matmul(
                                ps[g][:], lhs(hb, k, g), rhs(n, k),
                                start=(k == 0), stop=(k == KC - 1),
                            )
                else:
                    for k in range(KC):
                        for g in range(4):
                            nc.tensor.matmul(
                                ps[g][:], lhs(hb, k, g), rhs(n, k),
                                start=(k == 0), stop=(k == KC - 1),
                            )

                # ---- epilogue: evict PSUM via activation, cell update -----
                st = st_pool.tile([P, 6 * 512], f16, name="st")

                def sl(j):
                    return st[:, j * 512 : (j + 1) * 512]

                def gate_act(g, j):
                    m = hb * 4 + g
                    nc.scalar.activation(
                        sl(j), ps[g][:], ACT_FN[g], bias=bias_all[:, m : m + 1]
                    )

                gate_act(1, J_F)
                gate_act(0, J_I)
                gate_act(2, J_G)
                cell_update(st, hb, n, sl)
                th = sc_pool.tile([P, 512], f16, name="th")
                nc.scalar.activation(th[:], sl(J_CN), AF.Tanh)

                row = (n * 2 + hb) * P
                if final:
                    # bulk store (f,i,g,cn) while the o matmuls run
                    nc.scalar.dma_start(
                        out_ap[row : row + P, 0 : 4 * 512], st[:, 0 : 4 * 512]
                    )
                gate_act(3, J_O)
                nc.vector.tensor_mul(sl(J_HN), sl(J_O), th[:])

                if final:
                    nc.sync.dma_start(
                        out_ap[row : row + P, 4 * 512 : 5 * 512], sl(J_O)
                    )
                    nc.scalar.dma_start(
                        out_ap[row : row + P, 5 * 512 : 6 * 512], sl(J_HN)
                    )
                else:
                    nc.scalar.dma_start(out_ap[row : row + P, :], st[:])

    nc.compile()
    return nc


def _get_program():
    if "nc" not in _CACHE:
        _CACHE["nc"] = _build_program()
    return _CACHE["nc"]


def _gate_row_index(core: int) -> np.ndarray:
    """Global rows of W/b (4H-dim) owned by `core`, in m-tile order."""
    idx = []
    for hb in range(HB):
        for g in range(4):
            base = g * H + core * S + hb * P
            idx.extend(range(base, base + P))
    return np.asarray(idx)


def _build_in_maps(x, h_prev, C_prev, W_ih, b_ih, W_hh, b_hh):
    # xh_t: row (n*128 + p), col (k*512 + cc) -- element xh[n*512+cc, k*128+p]
    xh = np.concatenate([x, h_prev], axis=1)  # [B, K] fp32
    xh_t = np.ascontiguousarray(
        xh.reshape(NT, 512, KC, P).transpose(0, 3, 2, 1).reshape(NT * P, KC * 512)
    ).astype(_F16)
    bias_full = (b_ih + b_hh).astype(np.float32)

    in_maps = []
    for c in range(NCORES):
        idx = _gate_row_index(c)
        w_cat = np.concatenate([W_ih[idx], W_hh[idx]], axis=1)  # [1024, K]
        # w_t: row (hb*128 + p), col (k*512 + m') -- element
        # w_cat[hb*512 + m', k*128 + p]
        w_sw = (
            w_cat.reshape(HB, 512, KC, P)
            .transpose(0, 3, 2, 1)
            .reshape(HB * P, KC * 512)
        )
        c_slice = C_prev[:, c * S : (c + 1) * S]  # [B, 256]
        c_sw = c_slice.reshape(B, HB, P).transpose(2, 1, 0).reshape(P, HB * B)
        in_maps.append(
            {
                "w_t": np.ascontiguousarray(w_sw).astype(_F16),
                "xh_t": xh_t,
                "bias": np.ascontiguousarray(bias_full[idx].reshape(8, P).T),
                "c_t": np.ascontiguousarray(c_sw).astype(_F16),
            }
        )
    return in_maps


def kernel(x, h_prev, C_prev, W_ih, b_ih, W_hh, b_hh):
    from concourse.bass_utils import run_bass_kernel_spmd

    nc = _get_program()
    in_maps = _build_in_maps(x, h_prev, C_prev, W_ih, b_ih, W_hh, b_hh)
    _CACHE["last_in_maps"] = in_maps
    res = run_bass_kernel_spmd(nc, in_maps, core_ids=list(range(NCORES)))

    # out6: row ((n*2 + hb)*128 + p), col j*512 + cc
    blocks = []
    for c in range(NCORES):
        r = res.results[c]["out6"].reshape(NT, 2, P, 6, 512)
        # -> [j, (n, cc), (hb, p)]
        blocks.append(r.transpose(3, 0, 4, 1, 2).reshape(6, B, S))
    full = np.concatenate(blocks, axis=2).astype(np.float32)  # [6, B, H]
    return (
        full[J_HN],
        full[J_CN],
        full[J_F],
        full[J_I],
        full[J_G],
        full[J_O],
    )
